# revision 1
# baseline (speedup 1.0000x reference)
"""Trainium2 Bass kernel for nn_CustomPenaltyLayer (MinMax-inverse penalty loss).

Contract: kernel(**inputs) takes the FULL inputs (x:(1024,4096,8) f32,
min_:(8,), scale_:(8,)) and returns the FULL output (scalar f32), sharding
x row-wise across 8 NeuronCores internally.

Math (reference):
  x_inv = (x.reshape(-1, 8) - min_) / scale_
  d = x_inv[:, 2]; a = x_inv[:, 3]
  dev_pen   = count(~(0 <= d <= 252))
  act_pen   = count(a < 0 or a > 22)
  trans_pen = sum over adjacent pairs of [mod(prev,2)==0 & prev<20] *
              [(cur != prev+1) & (cur != 22)]
  num_act   = count(a != 22);  total = dev+act+trans + |num_act - 58|

Device strategy (per core, data-parallel rows). The kernel is DMA-bound
(~53 us to stream 16 MiB/core at ~340 GB/s), so compute is balanced
across ScalarE and VectorE to stay under the per-tile DMA time, and the
tile schedule tapers at the end so the last tile's compute tail is short.
  - DMA: x in [128, R_t, 8] f32 tiles, R_t = [1024,1024,1024,512,256,256].
  - ScalarE: a3 = (v3-min3)*rs3, a2 = (v2-min2)*rs2 (strided reads), and
    the two a3 range counts as sign-sums via activation accum_out
    (count(a<t) = (T - sum sign(a-t))/2 up to measure-zero boundary hits).
  - VectorE: count(a3 != 22) (shifted bf16 predicate output, so the
    transition product-sums hit the DVE 2x bf16 mode), the a2 range pair
    via clamp (count(clamp(a2,0,252) != a2)), and the transition term via
    the identity pen = cond*ne22s - cond*eq1 (the 3-factor product
    cond*(1-eq1)*ne22s reduces to it because cond & eq1 & (cur==22)
    requires prev==21, which is odd and fails cond). "a3 is an even
    integer" uses the 2^23 magic-number round trick (no mod ALU op).
  - Pairs spanning the R_t-row partition chunks are computed on the host;
    partial sums are combined on the host into the final scalar.
"""

import os
import sys

for _p in ("/opt/trn_rl_repo", os.path.expanduser("~/.axon_site/_ro/trn_rl_repo")):
    if os.path.isdir(_p) and _p not in sys.path:
        sys.path.append(_p)

import numpy as np

import concourse.bacc as bacc
import concourse.tile as tile
from concourse import mybir
from concourse.bass_utils import run_bass_kernel_spmd

F32 = mybir.dt.float32
BF16 = mybir.dt.bfloat16
ALU = mybir.AluOpType
ACTF = mybir.ActivationFunctionType

MAGIC = 8388608.0  # 2^23
BATCH, TIMESTEPS, D = 1024, 4096, 8
N_ROWS = BATCH * TIMESTEPS          # 4,194,304
N_CORES = 8
ROWS_PER_CORE = N_ROWS // N_CORES   # 524,288
P = 128                             # SBUF partitions
R_LIST = (256, 768, 1024, 1024, 512, 384, 128)   # rows/partition per tile
assert sum(R_LIST) * P == ROWS_PER_CORE
N_T = len(R_LIST)

_NC_CACHE = {}


def _build_nc(x_bufs: int = 3, work_bufs: int = 2):
    n_t = N_T
    nc = bacc.Bacc("TRN2", target_bir_lowering=False, debug=False)

    xs = nc.dram_tensor("xs", [ROWS_PER_CORE, 8], F32, kind="ExternalInput")
    consts = nc.dram_tensor("consts", [P, 8], F32, kind="ExternalInput")
    accS_d = nc.dram_tensor("accS", [P, 3 * n_t], F32, kind="ExternalOutput")
    accD_d = nc.dram_tensor("accD", [P, 3 * n_t], F32, kind="ExternalOutput")

    xs_flat = xs.ap()
    r_max = max(R_LIST)

    with tile.TileContext(nc) as tc:
        with (
            tc.tile_pool(name="xp", bufs=x_bufs) as xp,
            tc.tile_pool(name="ap_", bufs=3) as ap_pool,
            tc.tile_pool(name="wp", bufs=work_bufs) as wp,
            tc.tile_pool(name="acc", bufs=1) as accp,
        ):
            consts_sb = accp.tile([P, 8], F32, tag="consts")
            nc.sync.dma_start(consts_sb[:], consts.ap())
            # Absorb the consts-DMA wait into one dummy ACT op: the HW
            # Activation encoding has a single sync-wait slot, and the
            # loop's first ACT op must wait on the x-tile DMA instead.
            dummy = accp.tile([P, 1], F32, tag="dummy")
            nc.scalar.copy(dummy[:], consts_sb[:, 0:1])
            rs3 = consts_sb[:, 0:1]    # f32(1/scale3)
            b3 = consts_sb[:, 1:2]     # -min3*rs3
            rs2 = consts_sb[:, 2:3]    # f32(1/scale2)
            b2 = consts_sb[:, 3:4]     # -min2*rs2
            zero = consts_sb[:, 4:5]   # 0.0    (sign bias: a3 < 0 test)
            n22 = consts_sb[:, 5:6]    # -22.0  (sign bias: a3 > 22 test)

            accS = accp.tile([P, 3 * n_t], F32, tag="accS")   # ScalarE-owned
            accD = accp.tile([P, 3 * n_t], F32, tag="accD")   # VectorE-owned
            sgn = accp.tile([P, r_max], F32, tag="sgn")  # sign scratch

            off = 0
            for t, r in enumerate(R_LIST):
                x_t = xp.tile([P, r, 8], F32, tag="x")
                src = xs_flat[off:off + P * r, :].rearrange(
                    "(p r) d -> p r d", r=r)
                if r >= 512:  # split large transfers: slightly better overlap
                    h = r // 2
                    nc.sync.dma_start(x_t[:, :h, :], src[:, :h, :])
                    nc.sync.dma_start(x_t[:, h:, :], src[:, h:, :])
                else:
                    nc.sync.dma_start(x_t[:], src)
                off += P * r
                v2 = x_t[:, :, 2]
                v3 = x_t[:, :, 3]

                # ScalarE: affine transforms, 2 sign-counts on a3, and
                # ne22 = Square(sign(a3-22)) whose accum is count(a3 != 22).
                a3 = ap_pool.tile([P, r], F32, tag="a3")
                nc.scalar.activation(a3[:], v3, ACTF.Identity, bias=b3, scale=rs3)
                s22 = ap_pool.tile([P, r], F32, tag="s22")
                nc.scalar.activation(s22[:], a3[:], ACTF.Sign, bias=n22,
                                     accum_out=accS[:, 3 * t + 1:3 * t + 2])
                ne22 = ap_pool.tile([P, r], F32, tag="ne22")
                nc.scalar.activation(ne22[:], s22[:], ACTF.Square,
                                     accum_out=accS[:, 3 * t + 2:3 * t + 3])
                a2 = ap_pool.tile([P, r], F32, tag="a2")
                nc.scalar.activation(a2[:], v2, ACTF.Identity, bias=b2, scale=rs2)
                nc.scalar.activation(sgn[:, :r], a3[:], ACTF.Sign, bias=zero,
                                     accum_out=accS[:, 3 * t + 0:3 * t + 1])

                # VectorE. accD cols per tile:
                #   0: count(a2 out of [0,252])   1: s1   2: s2
                c2 = accD[:, 3 * t + 0:3 * t + 1]
                c3 = accD[:, 3 * t + 1:3 * t + 2]
                c4 = accD[:, 3 * t + 2:3 * t + 3]

                h2 = wp.tile([P, r], F32, tag="h2")
                nc.vector.tensor_scalar(h2[:], a3[:], 0.5, MAGIC,
                                        ALU.mult, ALU.add)
                r2 = wp.tile([P, r], F32, tag="r2")
                nc.vector.tensor_scalar(r2[:], h2[:], MAGIC, 2.0,
                                        ALU.subtract, ALU.mult)
                meq = wp.tile([P, r], F32, tag="meq")
                nc.vector.tensor_tensor(meq[:], r2[:], a3[:], ALU.is_equal)
                cond = wp.tile([P, r], BF16, tag="cond")
                nc.vector.scalar_tensor_tensor(cond[:], a3[:], 20.0, meq[:],
                                               ALU.is_lt, ALU.mult)
                eq1 = wp.tile([P, r], BF16, tag="eq1")
                nc.vector.scalar_tensor_tensor(eq1[:, :r - 1], a3[:, :r - 1], 1.0,
                                               a3[:, 1:r], ALU.add, ALU.is_equal)
                junk = wp.tile([P, r], BF16, tag="junk")
                # s1 = sum(cond[:-1] * ne22[1:])
                nc.vector.scalar_tensor_tensor(junk[:, :r - 1], cond[:, :r - 1],
                                               0.0, ne22[:, 1:r],
                                               ALU.add, ALU.mult, accum_out=c3)
                # s2 = sum(eq1 * cond[:-1])
                nc.vector.scalar_tensor_tensor(junk[:, :r - 1], eq1[:, :r - 1],
                                               0.0, cond[:, :r - 1],
                                               ALU.add, ALU.mult, accum_out=c4)
                # dev: count(clamp(a2, 0, 252) != a2)
                cl2 = wp.tile([P, r], F32, tag="cl2")
                nc.vector.tensor_scalar(cl2[:], a2[:], 0.0, 252.0,
                                        ALU.max, ALU.min)
                jf = wp.tile([P, r], F32, tag="jf")
                nc.vector.scalar_tensor_tensor(jf[:], cl2[:], 0.0, a2[:],
                                               ALU.add, ALU.not_equal,
                                               accum_out=c2)

            nc.sync.dma_start(accS_d.ap(), accS[:])
            nc.sync.dma_start(accD_d.ap(), accD[:])

    nc.compile()
    return nc


def _make_consts(min_, scale_):
    m = np.asarray(min_, dtype=np.float64)
    s = np.asarray(scale_, dtype=np.float64)
    rs3 = np.float32(1.0) / np.float32(s[3])
    rs2 = np.float32(1.0) / np.float32(s[2])
    vals = np.array([
        np.float64(rs3),
        -np.float64(np.float32(m[3])) * np.float64(rs3),
        np.float64(rs2),
        -np.float64(np.float32(m[2])) * np.float64(rs2),
        0.0,
        -22.0,
        0.0,
        0.0,
    ], dtype=np.float64).astype(np.float32)
    return np.broadcast_to(vals, (P, 8)).copy()


def _run_device(x_flat, min_, scale_, trace=False):
    if "nc" not in _NC_CACHE:
        _NC_CACHE["nc"] = _build_nc()
    nc = _NC_CACHE["nc"]
    consts = _make_consts(min_, scale_)
    in_maps = [
        {"xs": x_flat[c * ROWS_PER_CORE:(c + 1) * ROWS_PER_CORE], "consts": consts}
        for c in range(N_CORES)
    ]
    return run_bass_kernel_spmd(nc, in_maps, list(range(N_CORES)), trace=trace)


def _chunk_last_rows():
    """Global indices g of rows that END an R_t partition chunk (boundary
    pairs (g, g+1) are computed on the host). Excludes the final row."""
    gs = []
    for c in range(N_CORES):
        base = c * ROWS_PER_CORE
        off = 0
        for r in R_LIST:
            p = np.arange(P)
            gs.append(base + off + (p + 1) * r - 1)
            off += P * r
    g = np.concatenate(gs)
    return np.sort(g)[:-1]


def kernel(x, min_, scale_, _trace=False, _return_bkr=False):
    x = np.asarray(x, dtype=np.float32)
    min_ = np.asarray(min_, dtype=np.float32)
    scale_ = np.asarray(scale_, dtype=np.float32)
    x_flat = np.ascontiguousarray(x.reshape(-1, D))

    bkr = _run_device(x_flat, min_, scale_, trace=_trace)
    results = bkr.results

    T = float(ROWS_PER_CORE)
    dev = 0.0
    act = 0.0
    numact = 0.0
    trans = 0.0
    for c in range(N_CORES):
        res = results[c]
        aS = res["accS"].astype(np.float64).reshape(P, -1, 3).sum(axis=(0, 1))
        S_a3lo, S_a3hi, cnt_ne22 = aS
        aD = res["accD"].astype(np.float64).reshape(P, -1, 3).sum(axis=(0, 1))
        cnt_dev, s1, s2 = aD
        act += (T - S_a3lo) / 2.0 + (T + S_a3hi) / 2.0
        dev += cnt_dev
        numact += cnt_ne22
        trans += s1 - s2

    # host-side boundary pairs spanning partition chunks
    g = _chunk_last_rows()
    x3 = x_flat[:, 3]
    m3, s3 = min_[3], scale_[3]
    pa = ((x3[g] - m3) / s3).astype(np.float32)
    ca = ((x3[g + 1] - m3) / s3).astype(np.float32)
    cond = (np.mod(pa, np.float32(2.0)) == 0.0) & (pa < 20.0)
    invalid = (ca != pa + np.float32(1.0)) & (ca != np.float32(22.0))
    trans += np.where(cond, invalid.astype(np.float64), 0.0).sum()

    # Reproduce the reference's f32 summation order exactly.
    t1 = np.float32(dev)
    t2 = np.float32(act)
    t3 = np.float32(trans)
    t4 = np.float32(abs(numact - 58.0))
    out = np.array(((t1 + t2) + t3) + t4, dtype=np.float32)
    if _return_bkr:
        return out, bkr
    return out



# revision 5
# speedup vs baseline: 1.4548x; 1.4548x over previous
"""Trainium2 Bass kernel for nn_CustomPenaltyLayer (MinMax-inverse penalty loss).

Contract: kernel(**inputs) takes the FULL inputs (x:(1024,4096,8) f32,
min_:(8,), scale_:(8,)) and returns the FULL output (scalar f32), sharding
x row-wise across 8 NeuronCores internally.

Math (reference):
  x_inv = (x.reshape(-1, 8) - min_) / scale_
  d = x_inv[:, 2]; a = x_inv[:, 3]
  dev_pen   = count(~(0 <= d <= 252))
  act_pen   = count(a < 0) + count(a > 22)
  trans_pen = sum over adjacent pairs of [mod(prev,2)==0 & prev<20] *
              [(cur != prev+1) & (cur != 22)]
  num_act   = count(a != 22);  total = dev+act+trans + |num_act - 58|

Only columns 2 and 3 of x are used, so the host slices them out and ships
2 contiguous column arrays to the device (4 MiB/core instead of 16 MiB/core,
4x less HBM traffic; the device still streams every element it needs).

Device work per core (P=128 partitions, 4096 elems/partition, tiled):
  ScalarE : a3 = x3*rs3 + b3 (affine), h = a3*0.5 + 2^23, r2 = 2h - 2^24
            (magic-number round-to-nearest-even), S0 += sum sign(a3).
  Pool    : t1 = a3 - r2 (signed distance to nearest even int), u = |t1|.
  VectorE : C_ev += count(u < tau)   [rare-event detector, see below]
            C_hi += count(a3 > 22)
            cl = clamp(x2, m2, X252); D += count(cl != x2)  [dev_pen, exact:
            aref2 < 0 iff x2 < m2; aref2 > 252 iff x2 > X252 (host-bisected)]

Exactness strategy: all terms that depend on float-rounding boundary cases
(a == 0, a == 22, a == even integer for the transition term) can only
disagree between the device's (x-m)*rs rounding and the reference's (x-m)/s
rounding when a3 lands within a few ulp of an even integer (0 and 22 are
even). Those elements ALWAYS satisfy |a3 - nearest_even(a3)| < tau=2^-14
(worst-case rounding gap is a few hundred ulp < 2.5e-5 for |a|<=128), so
they land in detector-flagged (tile, partition) cells. The host re-scans
flagged cells with exact reference semantics (np.float32 division) and
replaces those cells' counts; unflagged cells are provably exact as-is.
The transition penalty comes entirely from cond-hits (mod(prev,2)==0),
which only occur in flagged cells, so the host computes it exactly there
(successor element read globally - no separate boundary-pair handling).
"""

import os
import sys

for _p in ("/opt/trn_rl_repo", os.path.expanduser("~/.axon_site/_ro/trn_rl_repo")):
    if os.path.isdir(_p) and _p not in sys.path:
        sys.path.append(_p)

import numpy as np

import concourse.bacc as bacc
import concourse.tile as tile
from concourse import mybir
from concourse.bass_utils import run_bass_kernel_spmd

F32 = mybir.dt.float32
ALU = mybir.AluOpType
ACTF = mybir.ActivationFunctionType

MAGIC = 8388608.0                   # 2^23
TAU = 2.0 ** -14
BATCH, TIMESTEPS, D = 1024, 4096, 8
N_ROWS = BATCH * TIMESTEPS          # 4,194,304
N_CORES = 8
ROWS_PER_CORE = N_ROWS // N_CORES   # 524,288
P = 128                             # SBUF partitions
R_LIST = (1024, 1024, 1024, 768, 256)   # elems/partition per tile
assert sum(R_LIST) * P == ROWS_PER_CORE
N_T = len(R_LIST)

_NC_CACHE = {}


def _build_nc():
    nc = bacc.Bacc("TRN2", target_bir_lowering=False, debug=False)

    xs3 = nc.dram_tensor("xs3", [ROWS_PER_CORE], F32, kind="ExternalInput")
    xs2 = nc.dram_tensor("xs2", [ROWS_PER_CORE], F32, kind="ExternalInput")
    consts = nc.dram_tensor("consts", [P, 12], F32, kind="ExternalInput")
    accA_d = nc.dram_tensor("accA", [P, N_T], F32, kind="ExternalOutput")
    accV_d = nc.dram_tensor("accV", [P, 3 * N_T], F32, kind="ExternalOutput")

    x3_flat = xs3.ap()
    x2_flat = xs2.ap()

    with tile.TileContext(nc) as tc:
        with (
            tc.tile_pool(name="x3p", bufs=3) as x3p,
            tc.tile_pool(name="x2p", bufs=3) as x2p,
            tc.tile_pool(name="ap_", bufs=2) as ap_pool,
            tc.tile_pool(name="pp", bufs=2) as pp,
            tc.tile_pool(name="wp", bufs=2) as wp,
            tc.tile_pool(name="acc", bufs=1) as accp,
        ):
            cn = accp.tile([P, 12], F32, tag="consts")
            nc.sync.dma_start(cn[:], consts.ap())
            # Absorb the consts-DMA wait into one dummy ACT op so the loop's
            # first ACT op waits on the x-tile DMA instead (single wait slot).
            dummy = accp.tile([P, 1], F32, tag="dummy")
            nc.scalar.copy(dummy[:], cn[:, 0:1])
            rs3 = cn[:, 0:1]    # f32(1/scale3)
            b3 = cn[:, 1:2]     # -min3*rs3
            half = cn[:, 2:3]   # 0.5
            mg = cn[:, 3:4]     # 2^23
            two = cn[:, 4:5]    # 2.0
            n2mg = cn[:, 5:6]   # -2^24
            zero = cn[:, 6:7]   # 0.0
            m2 = cn[:, 7:8]     # min2 (dev lower threshold, x2-space)
            x252 = cn[:, 8:9]   # bisected upper threshold (x2-space)

            accA = accp.tile([P, N_T], F32, tag="accA")       # ScalarE S0
            accV = accp.tile([P, 3 * N_T], F32, tag="accV")   # DVE C_ev,C_hi,D

            off = 0
            for t, r in enumerate(R_LIST):
                x3t = x3p.tile([P, r], F32, tag="x3")
                nc.sync.dma_start(
                    x3t[:], x3_flat[off:off + P * r].rearrange("(p r) -> p r", r=r))
                x2t = x2p.tile([P, r], F32, tag="x2")
                nc.sync.dma_start(
                    x2t[:], x2_flat[off:off + P * r].rearrange("(p r) -> p r", r=r))
                off += P * r

                # ScalarE
                a3 = ap_pool.tile([P, r], F32, tag="a3")
                nc.scalar.activation(a3[:], x3t[:], ACTF.Identity,
                                     bias=b3, scale=rs3)
                h = ap_pool.tile([P, r], F32, tag="h")
                nc.scalar.activation(h[:], a3[:], ACTF.Identity,
                                     bias=mg, scale=half)
                r2 = ap_pool.tile([P, r], F32, tag="r2")
                nc.scalar.activation(r2[:], h[:], ACTF.Identity,
                                     bias=n2mg, scale=two)
                sg0 = wp.tile([P, r], F32, tag="sg0")
                nc.scalar.activation(sg0[:], a3[:], ACTF.Sign, bias=zero,
                                     accum_out=accA[:, t:t + 1])

                # Pool
                t1 = pp.tile([P, r], F32, tag="t1")
                nc.gpsimd.tensor_tensor(t1[:], a3[:], r2[:], ALU.subtract)
                u = pp.tile([P, r], F32, tag="u")
                nc.gpsimd.tensor_tensor(u[:], t1[:], t1[:], ALU.mult)

                # VectorE
                ju = wp.tile([P, r], F32, tag="ju")
                nc.vector.tensor_scalar(ju[:], u[:], TAU * TAU, None,
                                        ALU.is_lt, ALU.add,
                                        accum_out=accV[:, 3 * t:3 * t + 1])
                j22 = wp.tile([P, r], F32, tag="j22")
                nc.vector.tensor_scalar(j22[:], a3[:], 22.0, None,
                                        ALU.is_gt, ALU.add,
                                        accum_out=accV[:, 3 * t + 1:3 * t + 2])
                cl = wp.tile([P, r], F32, tag="cl")
                nc.vector.tensor_scalar(cl[:], x2t[:], m2, x252,
                                        ALU.max, ALU.min)
                jne = wp.tile([P, r], F32, tag="jne")
                nc.vector.scalar_tensor_tensor(jne[:], cl[:], 0.0, x2t[:],
                                               ALU.add, ALU.not_equal,
                                               accum_out=accV[:, 3 * t + 2:3 * t + 3])

            nc.sync.dma_start(accA_d.ap(), accA[:])
            nc.sync.dma_start(accV_d.ap(), accV[:])

    nc.compile()
    return nc


def _f32(v):
    return np.float32(v)


def _bisect_upper(m, s, lim):
    """Largest f32 v with f32((v - m)/s) <= lim (monotone in v; exact)."""
    m = _f32(m)
    s = _f32(s)
    lim = _f32(lim)

    def f(v):
        with np.errstate(over="ignore"):
            return _f32((_f32(v) - m) / s)

    hi = np.finfo(np.float32).max
    if f(hi) <= lim:
        return hi
    lo = m                      # f(m) == 0 <= lim
    assert f(lo) <= lim
    lo_b = int(lo.view(np.uint32))
    hi_b = int(hi.view(np.uint32))
    # positive floats: bit pattern order == value order
    while hi_b - lo_b > 1:
        mid_b = (lo_b + hi_b) // 2
        v = np.uint32(mid_b).view(np.float32)
        if f(v) <= lim:
            lo_b = mid_b
        else:
            hi_b = mid_b
    return np.uint32(lo_b).view(np.float32)


def _make_consts(min_, scale_):
    m3 = _f32(min_[3])
    s3 = _f32(scale_[3])
    m2 = _f32(min_[2])
    s2 = _f32(scale_[2])
    rs3 = _f32(1.0) / s3
    b3 = _f32(-np.float64(m3) * np.float64(rs3))
    x252 = _bisect_upper(m2, s2, 252.0)
    vals = np.array([rs3, b3, 0.5, MAGIC, 2.0, -2.0 * MAGIC, 0.0,
                     m2, x252, 0.0, 0.0, 0.0], dtype=np.float32)
    return np.broadcast_to(vals, (P, 12)).copy()


def _run_device(x3col, x2col, min_, scale_, trace=False):
    if "nc" not in _NC_CACHE:
        _NC_CACHE["nc"] = _build_nc()
    nc = _NC_CACHE["nc"]
    consts = _make_consts(min_, scale_)
    in_maps = [
        {"xs3": x3col[c * ROWS_PER_CORE:(c + 1) * ROWS_PER_CORE],
         "xs2": x2col[c * ROWS_PER_CORE:(c + 1) * ROWS_PER_CORE],
         "consts": consts}
        for c in range(N_CORES)
    ]
    return run_bass_kernel_spmd(nc, in_maps, list(range(N_CORES)), trace=trace)


def kernel(x, min_, scale_, _trace=False, _return_bkr=False):
    x = np.asarray(x, dtype=np.float32)
    min_ = np.asarray(min_, dtype=np.float32)
    scale_ = np.asarray(scale_, dtype=np.float32)
    xr = x.reshape(-1, D)
    x3col = np.ascontiguousarray(xr[:, 3])
    x2col = np.ascontiguousarray(xr[:, 2])

    bkr = _run_device(x3col, x2col, min_, scale_, trace=_trace)
    results = bkr.results

    m3 = _f32(min_[3])
    s3 = _f32(scale_[3])

    # Per-cell accumulators, cells indexed (core, tile, partition)
    dev = 0.0
    act_lo = 0.0
    act_hi = 0.0
    eq22 = 0.0
    trans = 0.0
    tile_base = np.cumsum([0] + [P * r for r in R_LIST])[:-1]

    for c in range(N_CORES):
        res = results[c]
        S0 = res["accA"].astype(np.float64)                  # [P, N_T]
        aV = res["accV"].astype(np.float64).reshape(P, N_T, 3)
        C_ev = aV[:, :, 0]
        C_hi = aV[:, :, 1]
        dev += aV[:, :, 2].sum()

        flagged = C_ev > 0.0                                  # [P, N_T]
        r_arr = np.array(R_LIST, dtype=np.float64)[None, :]   # [1, N_T]
        unflag = ~flagged
        act_lo += (((r_arr - S0) * 0.5) * unflag).sum()
        act_hi += (C_hi * unflag).sum()

        # exact host re-scan of flagged cells with reference semantics
        ps, ts_ = np.nonzero(flagged)
        for p, t in zip(ps, ts_):
            r = R_LIST[t]
            start = c * ROWS_PER_CORE + tile_base[t] + p * r
            xs = x3col[start:start + r]
            aref = ((xs - m3) / s3).astype(np.float32)
            act_lo += float((aref < 0).sum())
            act_hi += float((aref > 22.0).sum())
            eq22 += float((aref == np.float32(22.0)).sum())
            cond = (np.mod(aref, np.float32(2.0)) == 0.0) & (aref < 20.0)
            for j in np.nonzero(cond)[0]:
                i = start + int(j)
                if i + 1 >= N_ROWS:
                    continue
                an = _f32((_f32(x3col[i + 1]) - m3) / s3)
                ap_ = aref[j]
                if (an != ap_ + np.float32(1.0)) and (an != np.float32(22.0)):
                    trans += 1.0

    numact = float(N_ROWS) - eq22
    act = act_lo + act_hi

    # Reproduce the reference's f32 summation order exactly.
    t1 = np.float32(dev)
    t2 = np.float32(act)
    t3 = np.float32(trans)
    t4 = np.float32(abs(numact - 58.0))
    out = np.array(((t1 + t2) + t3) + t4, dtype=np.float32)
    if _return_bkr:
        return out, bkr
    return out


# revision 7
# speedup vs baseline: 1.6213x; 1.1144x over previous
"""Trainium2 Bass kernel for nn_CustomPenaltyLayer (MinMax-inverse penalty loss).

Contract: kernel(**inputs) takes the FULL inputs (x:(1024,4096,8) f32,
min_:(8,), scale_:(8,)) and returns the FULL output (scalar f32), sharding
x row-wise across 8 NeuronCores internally.

Math (reference):
  x_inv = (x.reshape(-1, 8) - min_) / scale_
  d = x_inv[:, 2]; a = x_inv[:, 3]
  dev_pen   = count(~(0 <= d <= 252))
  act_pen   = count(a < 0) + count(a > 22)
  trans_pen = sum over adjacent pairs of [mod(prev,2)==0 & prev<20] *
              [(cur != prev+1) & (cur != 22)]
  num_act   = count(a != 22);  total = dev+act+trans + |num_act - 58|

Only columns 2 and 3 of x are used, so the host slices them out and ships
2 contiguous column arrays to the device (4 MiB/core instead of 16 MiB/core,
4x less HBM traffic; the device still streams every element it needs).

Device work per core (P=128 partitions, 4096 elems/partition, tiled):
  ScalarE : a3 = x3*rs3 + b3 (affine), h = a3*0.5 + 2^23, r2 = 2h - 2^24
            (magic-number round-to-nearest-even), S0 += sum sign(a3).
  Pool    : t1 = a3 - r2 (signed distance to nearest even int), u = |t1|.
  VectorE : C_ev += count(u < tau)   [rare-event detector, see below]
            C_hi += count(a3 > 22)
            cl = clamp(x2, m2, X252); D += count(cl != x2)  [dev_pen, exact:
            aref2 < 0 iff x2 < m2; aref2 > 252 iff x2 > X252 (host-bisected)]

Exactness strategy: all terms that depend on float-rounding boundary cases
(a == 0, a == 22, a == even integer for the transition term) can only
disagree between the device's (x-m)*rs rounding and the reference's (x-m)/s
rounding when a3 lands within a few ulp of an even integer (0 and 22 are
even). Those elements ALWAYS satisfy |a3 - nearest_even(a3)| < tau=2^-14
(worst-case rounding gap is a few hundred ulp < 2.5e-5 for |a|<=128), so
they land in detector-flagged (tile, partition) cells. The host re-scans
flagged cells with exact reference semantics (np.float32 division) and
replaces those cells' counts; unflagged cells are provably exact as-is.
The transition penalty comes entirely from cond-hits (mod(prev,2)==0),
which only occur in flagged cells, so the host computes it exactly there
(successor element read globally - no separate boundary-pair handling).
"""

import os
import sys

for _p in ("/opt/trn_rl_repo", os.path.expanduser("~/.axon_site/_ro/trn_rl_repo")):
    if os.path.isdir(_p) and _p not in sys.path:
        sys.path.append(_p)

import numpy as np

import concourse.bacc as bacc
import concourse.tile as tile
from concourse import mybir
from concourse.bass_utils import run_bass_kernel_spmd

F32 = mybir.dt.float32
BF16 = mybir.dt.bfloat16
ALU = mybir.AluOpType
ACTF = mybir.ActivationFunctionType

MAGIC = 8388608.0                   # 2^23
TAU = 2.0 ** -14
BATCH, TIMESTEPS, D = 1024, 4096, 8
N_ROWS = BATCH * TIMESTEPS          # 4,194,304
N_CORES = 8
ROWS_PER_CORE = N_ROWS // N_CORES   # 524,288
P = 128                             # SBUF partitions
R_LIST = (1024, 1024, 1024, 768, 256)   # elems/partition per tile
assert sum(R_LIST) * P == ROWS_PER_CORE
N_T = len(R_LIST)

_NC_CACHE = {}


def _build_nc():
    nc = bacc.Bacc("TRN2", target_bir_lowering=False, debug=False)

    xs3 = nc.dram_tensor("xs3", [ROWS_PER_CORE], F32, kind="ExternalInput")
    xs2 = nc.dram_tensor("xs2", [ROWS_PER_CORE], F32, kind="ExternalInput")
    consts = nc.dram_tensor("consts", [P, 12], F32, kind="ExternalInput")
    accV_d = nc.dram_tensor("accV", [P, 4 * N_T], F32, kind="ExternalOutput")

    x3_flat = xs3.ap()
    x2_flat = xs2.ap()

    with tile.TileContext(nc) as tc:
        with (
            tc.tile_pool(name="x3p", bufs=3) as x3p,
            tc.tile_pool(name="x2p", bufs=3) as x2p,
            tc.tile_pool(name="ap_", bufs=2) as ap_pool,
            tc.tile_pool(name="pp", bufs=2) as pp,
            tc.tile_pool(name="wp", bufs=2) as wp,
            tc.tile_pool(name="acc", bufs=1) as accp,
        ):
            cn = accp.tile([P, 12], F32, tag="consts")
            nc.sync.dma_start(cn[:], consts.ap())
            # Absorb the consts-DMA wait into one dummy ACT op so the loop's
            # first ACT op waits on the x-tile DMA instead (single wait slot).
            dummy = accp.tile([P, 1], F32, tag="dummy")
            nc.scalar.copy(dummy[:], cn[:, 0:1])
            rs3 = cn[:, 0:1]    # f32(1/scale3)
            bw = cn[:, 1:2]     # -min3*rs3 - 11  (w = a3 - 11)
            half = cn[:, 2:3]   # 0.5
            m05 = cn[:, 3:4]    # 2^23 + 0.5 (odd-grid magic)
            two = cn[:, 4:5]    # 2.0
            nmg = cn[:, 5:6]    # -(2^24 + 1)
            zero = cn[:, 6:7]   # 0.0
            m2 = cn[:, 7:8]     # min2 (dev lower threshold, x2-space)
            x252 = cn[:, 8:9]   # bisected upper threshold (x2-space)

            accV = accp.tile([P, 4 * N_T], F32, tag="accV")   # C_ev,C_out,D_lo,D_hi

            off = 0
            for t, r in enumerate(R_LIST):
                x3t = x3p.tile([P, r], F32, tag="x3")
                nc.sync.dma_start(
                    x3t[:], x3_flat[off:off + P * r].rearrange("(p r) -> p r", r=r))
                x2t = x2p.tile([P, r], F32, tag="x2")
                nc.sync.dma_start(
                    x2t[:], x2_flat[off:off + P * r].rearrange("(p r) -> p r", r=r))
                off += P * r

                # ScalarE: w = a3 - 11; h2/r2p = magic round of w to the
                # nearest odd integer (even integers of a3); w2 = w^2.
                w = ap_pool.tile([P, r], F32, tag="w")
                nc.scalar.activation(w[:], x3t[:], ACTF.Identity,
                                     bias=bw, scale=rs3)
                h2 = ap_pool.tile([P, r], F32, tag="h2")
                nc.scalar.activation(h2[:], w[:], ACTF.Identity,
                                     bias=m05, scale=half)
                r2p = ap_pool.tile([P, r], F32, tag="r2p")
                nc.scalar.activation(r2p[:], h2[:], ACTF.Identity,
                                     bias=nmg, scale=two)
                w2 = ap_pool.tile([P, r], F32, tag="w2")
                nc.scalar.activation(w2[:], w[:], ACTF.Square, bias=zero)

                # Pool: t1 = distance of w to nearest odd int; u = t1^2.
                # bf16 is plenty for the tau-window detector (5x margin).
                t1 = pp.tile([P, r], BF16, tag="t1")
                nc.gpsimd.tensor_tensor(t1[:], w[:], r2p[:], ALU.subtract)
                u = pp.tile([P, r], BF16, tag="u")
                nc.gpsimd.tensor_tensor(u[:], t1[:], t1[:], ALU.mult)

                # VectorE: 4 single-op counts
                ju = wp.tile([P, r], BF16, tag="ju")
                nc.vector.tensor_scalar(ju[:], u[:], TAU * TAU, None,
                                        ALU.is_lt, ALU.add,
                                        accum_out=accV[:, 4 * t:4 * t + 1])
                jout = wp.tile([P, r], F32, tag="jout")
                nc.vector.tensor_scalar(jout[:], w2[:], 121.0, None,
                                        ALU.is_gt, ALU.add,
                                        accum_out=accV[:, 4 * t + 1:4 * t + 2])
                jdlo = wp.tile([P, r], F32, tag="jdlo")
                nc.vector.tensor_scalar(jdlo[:], x2t[:], m2, None,
                                        ALU.is_lt, ALU.add,
                                        accum_out=accV[:, 4 * t + 2:4 * t + 3])
                jdhi = wp.tile([P, r], F32, tag="jdhi")
                nc.vector.tensor_scalar(jdhi[:], x2t[:], x252, None,
                                        ALU.is_gt, ALU.add,
                                        accum_out=accV[:, 4 * t + 3:4 * t + 4])

            nc.sync.dma_start(accV_d.ap(), accV[:])

    nc.compile()
    return nc


def _f32(v):
    return np.float32(v)


def _bisect_upper(m, s, lim):
    """Largest f32 v with f32((v - m)/s) <= lim (monotone in v; exact)."""
    m = _f32(m)
    s = _f32(s)
    lim = _f32(lim)

    def f(v):
        with np.errstate(over="ignore"):
            return _f32((_f32(v) - m) / s)

    hi = np.finfo(np.float32).max
    if f(hi) <= lim:
        return hi
    lo = m                      # f(m) == 0 <= lim
    assert f(lo) <= lim
    lo_b = int(lo.view(np.uint32))
    hi_b = int(hi.view(np.uint32))
    # positive floats: bit pattern order == value order
    while hi_b - lo_b > 1:
        mid_b = (lo_b + hi_b) // 2
        v = np.uint32(mid_b).view(np.float32)
        if f(v) <= lim:
            lo_b = mid_b
        else:
            hi_b = mid_b
    return np.uint32(lo_b).view(np.float32)


def _make_consts(min_, scale_):
    m3 = _f32(min_[3])
    s3 = _f32(scale_[3])
    m2 = _f32(min_[2])
    s2 = _f32(scale_[2])
    rs3 = _f32(1.0) / s3
    b3 = _f32(-np.float64(m3) * np.float64(rs3))
    bw = _f32(np.float64(b3) - 11.0)
    x252 = _bisect_upper(m2, s2, 252.0)
    vals = np.array([rs3, bw, 0.5, MAGIC + 0.5, 2.0, -(2.0 * MAGIC + 1.0),
                     0.0, m2, x252, 0.0, 0.0, 0.0], dtype=np.float32)
    return np.broadcast_to(vals, (P, 12)).copy()


def _run_device(x3col, x2col, min_, scale_, trace=False):
    if "nc" not in _NC_CACHE:
        _NC_CACHE["nc"] = _build_nc()
    nc = _NC_CACHE["nc"]
    consts = _make_consts(min_, scale_)
    in_maps = [
        {"xs3": x3col[c * ROWS_PER_CORE:(c + 1) * ROWS_PER_CORE],
         "xs2": x2col[c * ROWS_PER_CORE:(c + 1) * ROWS_PER_CORE],
         "consts": consts}
        for c in range(N_CORES)
    ]
    return run_bass_kernel_spmd(nc, in_maps, list(range(N_CORES)), trace=trace)


def kernel(x, min_, scale_, _trace=False, _return_bkr=False):
    x = np.asarray(x, dtype=np.float32)
    min_ = np.asarray(min_, dtype=np.float32)
    scale_ = np.asarray(scale_, dtype=np.float32)
    xr = x.reshape(-1, D)
    x3col = np.ascontiguousarray(xr[:, 3])
    x2col = np.ascontiguousarray(xr[:, 2])

    bkr = _run_device(x3col, x2col, min_, scale_, trace=_trace)
    results = bkr.results

    m3 = _f32(min_[3])
    s3 = _f32(scale_[3])

    # Per-cell accumulators, cells indexed (core, tile, partition)
    dev = 0.0
    act_lo = 0.0
    act_hi = 0.0
    eq22 = 0.0
    trans = 0.0
    tile_base = np.cumsum([0] + [P * r for r in R_LIST])[:-1]

    for c in range(N_CORES):
        res = results[c]
        aV = res["accV"].astype(np.float64).reshape(P, N_T, 4)
        C_ev = aV[:, :, 0]
        C_out = aV[:, :, 1]
        dev += aV[:, :, 2].sum() + aV[:, :, 3].sum()

        flagged = C_ev > 0.0                                  # [P, N_T]
        unflag = ~flagged
        act_lo += (C_out * unflag).sum()

        # exact host re-scan of flagged cells with reference semantics
        ps, ts_ = np.nonzero(flagged)
        for p, t in zip(ps, ts_):
            r = R_LIST[t]
            start = c * ROWS_PER_CORE + tile_base[t] + p * r
            xs = x3col[start:start + r]
            aref = ((xs - m3) / s3).astype(np.float32)
            act_lo += float(((aref < 0) | (aref > 22.0)).sum())
            eq22 += float((aref == np.float32(22.0)).sum())
            cond = (np.mod(aref, np.float32(2.0)) == 0.0) & (aref < 20.0)
            for j in np.nonzero(cond)[0]:
                i = start + int(j)
                if i + 1 >= N_ROWS:
                    continue
                an = _f32((_f32(x3col[i + 1]) - m3) / s3)
                ap_ = aref[j]
                if (an != ap_ + np.float32(1.0)) and (an != np.float32(22.0)):
                    trans += 1.0

    numact = float(N_ROWS) - eq22
    act = act_lo + act_hi

    # Reproduce the reference's f32 summation order exactly.
    t1 = np.float32(dev)
    t2 = np.float32(act)
    t3 = np.float32(trans)
    t4 = np.float32(abs(numact - 58.0))
    out = np.array(((t1 + t2) + t3) + t4, dtype=np.float32)
    if _return_bkr:
        return out, bkr
    return out
